# revision 18
# baseline (speedup 1.0000x reference)
"""Trainium2 Bass kernel for the L1 tensor-product problem.

Math (per batch row b):
  out0e = [x0e*s, CG*(x1o.v)] @ W0e * NORM0E
  out0o = [x0o*s, CG*(x1e.v)] @ W0o * NORM0O
  out1e_c = [CG*x0o*v_c, CG*x1e_c*s, CGC*cross(x1o,v)_c] @ W1e * NORM1E
  out1o_c = [CG*x0e*v_c, CG*x1o_c*s, CGC*cross(x1e,v)_c] @ W1o * NORM1O

Kernel strategy (pure data parallel over batch, 8 cores), v2:
  * Everything bf16 on the wire and on the matmul path; PSUM accumulates
    fp32; output is written bf16 and upcast on the host (rel-err budget
    2e-2, bf16 gives ~3e-3).
  * Host packs x per core as [ntiles, 128, 10*T] so each partition's
    tile-load is one contiguous 10KB DMA descriptor (the fp32 baseline
    was descriptor-bound at ~2KB/descriptor, DMA 95% busy).
  * The per-row scalars (s, v_c) commute with the feature contraction:
    the x0o@W1e / x0e@W1o blocks are computed UNSCALED (2 matmuls each
    instead of 6 pre-scaled ones) and scaled on DVE afterwards.  This
    cuts matmuls per tile from 54 to 42.
  * The 40 pre-scaled products are emitted as 4 wide DVE instructions
    (x_all * s broadcast; x1e/x1o * v_c broadcast) using stride-0
    broadcast APs; multiplier rows are partition-broadcast by GpSimd,
    keeping the PE free of the old ones-matmul broadcasts.
"""

import sys

sys.path.insert(0, "/opt/trn_rl_repo")

import numpy as np

import concourse.bass as bass
import concourse.bacc as bacc
import concourse.mybir as mybir
from concourse.bass_utils import run_bass_kernel_spmd
from concourse.tile import TileContext

N_CORES = 8
T = 512  # batch columns per tile

# irreps: 256x0e + 256x0o + 128x1e + 128x1o
CG = 1.0 / 3.0**0.5
CGC = 1.0 / 6.0**0.5
NORM0E = (1.0 / 384.0) ** 0.5
NORM0O = (1.0 / 384.0) ** 0.5
NORM1E = (3.0 / 512.0) ** 0.5
NORM1O = (3.0 / 512.0) ** 0.5

_BF16 = None


def _bf16():
    global _BF16
    if _BF16 is None:
        import ml_dtypes

        _BF16 = np.dtype(ml_dtypes.bfloat16)
    return _BF16


def _pack_weights(W0e, W0o, W1e, W1o):
    """Fold constants/signs; 22 lhsT chunks [128,128] side by side.

    Order: 0e (kc0m0,kc0m1,kc1m0,kc1m1,kc2m0,kc2m1), 0o (same 6),
    1e (g0,g1,h,k+,k-), 1o (g0,g1,h,k+,k-).
    """
    W0e = W0e.astype(np.float64) * NORM0E
    W0e[256:] *= CG
    W0o = W0o.astype(np.float64) * NORM0O
    W0o[256:] *= CG
    W1e = W1e.astype(np.float64) * NORM1E
    W1e[:384] *= CG
    W1e[384:] *= CGC
    W1o = W1o.astype(np.float64) * NORM1O
    W1o[:384] *= CG
    W1o[384:] *= CGC
    chunks = []
    for W in (W0e, W0o):  # [384, 256]
        for kc in range(3):
            for mc in range(2):
                chunks.append(W[kc * 128 : (kc + 1) * 128, mc * 128 : (mc + 1) * 128])
    for W in (W1e, W1o):  # [512, 128]
        chunks.append(W[0:128, :])      # g0
        chunks.append(W[128:256, :])    # g1
        chunks.append(W[256:384, :])    # h
        chunks.append(W[384:512, :])    # k+
        chunks.append(-W[384:512, :])   # k-
    chunks.append(np.ones((128, 128), np.float64))  # 22: ones row for bcast
    packed = np.concatenate(chunks, axis=1)
    return np.ascontiguousarray(packed.astype(_bf16()))


def _prep_shard(in1_s, in2_s):
    """in1 [Bs,1280] -> x [nt, 128, 10*T] bf16; in2 [Bs,4] -> s4 [nt,4,T].

    Chunk order: 0,1=x0e  2,3=x0o  4+c=x1e_c  7+c=x1o_c.
    """
    Bs = in1_s.shape[0]
    nt = Bs // T
    dt = _bf16()
    x = np.empty((nt, 128, 10, T), dt)
    x[:, :, 0:4] = in1_s[:, 0:512].reshape(nt, T, 4, 128).transpose(0, 3, 2, 1)
    x[:, :, 4:7] = in1_s[:, 512:896].reshape(nt, T, 128, 3).transpose(0, 2, 3, 1)
    x[:, :, 7:10] = in1_s[:, 896:1280].reshape(nt, T, 128, 3).transpose(0, 2, 3, 1)
    s4 = np.ascontiguousarray(in2_s.reshape(nt, T, 4).transpose(0, 2, 1).astype(dt))
    return np.ascontiguousarray(x.reshape(nt, 128, 10 * T)), s4


def _post_shard(y):
    """Device y [nt, 128, 10*T] bf16 -> [Bs, 1280] fp32 original layout."""
    nt = y.shape[0]
    y = np.asarray(y).reshape(nt, 128, 10, T).astype(np.float32)
    out = np.empty((nt, T, 1280), np.float32)
    out[:, :, 0:512] = y[:, :, 0:4].transpose(0, 3, 2, 1).reshape(nt, T, 512)
    out[:, :, 512:896] = y[:, :, 4:7].transpose(0, 3, 1, 2).reshape(nt, T, 384)
    out[:, :, 896:1280] = y[:, :, 7:10].transpose(0, 3, 1, 2).reshape(nt, T, 384)
    return out.reshape(nt * T, 1280)


def _build_program(Bs):
    assert Bs % T == 0, (Bs, T)
    nt = Bs // T
    bf = mybir.dt.bfloat16
    f32 = mybir.dt.float32

    nc = bacc.Bacc()
    x = nc.declare_dram_parameter("x", [nt, 128, 10 * T], bf, isOutput=False)
    s4 = nc.declare_dram_parameter("s4", [nt, 4, T], bf, isOutput=False)
    w = nc.declare_dram_parameter("w", [128, 23 * 128], bf, isOutput=False)
    y = nc.declare_dram_parameter("y", [nt, 128, 10 * T], bf, isOutput=True)

    with TileContext(nc) as tc:
        with (
            tc.tile_pool(name="wpool", bufs=1) as wpool,
            tc.tile_pool(name="xpool", bufs=2) as xpool,
            tc.tile_pool(name="mbpool", bufs=2) as mbpool,
            tc.tile_pool(name="pspool", bufs=2) as pspool,
            tc.tile_pool(name="pvpool", bufs=6) as pvpool,
            tc.tile_pool(name="cpool", bufs=2) as cpool,
            tc.tile_pool(name="ypool", bufs=2) as ypool,
            tc.tile_pool(name="psum", bufs=8, space="PSUM") as psum,
        ):
            wt = wpool.tile([128, 23 * 128], bf)
            nc.sync.dma_start(out=wt[:, :], in_=w[:, :])

            def W(i):
                return wt[:, i * 128 : (i + 1) * 128]

            for t in range(nt):
                # ---- loads ----
                xt = xpool.tile([128, 10 * T], bf, tag="xt", name="x_t")
                nc.sync.dma_start(out=xt[:, :], in_=x[t, :, :])
                # ---- multiplier broadcast via stride-0 DMA ----
                mbt = mbpool.tile([128, 4 * T], bf, tag="mb", name="mb_t")
                nc.sync.dma_start(
                    out=mbt[:, :].rearrange("p (c t) -> p c t", c=4),
                    in_=s4[t].unsqueeze(0).broadcast_to([128, 4, T]),
                )

                def mb(j, nch):
                    # [128, nch, T] stride-0 broadcast of multiplier row j
                    return (
                        mbt[:, j * T : (j + 1) * T]
                        .unsqueeze(1)
                        .broadcast_to([128, nch, T])
                    )

                # ---- products: DVE bulk + GpSimd slices ----
                ps = pspool.tile([128, 10 * T], bf, tag="ps", name="ps_t")
                nc.vector.tensor_mul(
                    ps[:, : 8 * T].rearrange("p (c t) -> p c t", c=8),
                    xt[:, : 8 * T].rearrange("p (c t) -> p c t", c=8),
                    mb(0, 8),
                )
                nc.gpsimd.tensor_mul(
                    ps[:, 8 * T :].rearrange("p (c t) -> p c t", c=2),
                    xt[:, 8 * T :].rearrange("p (c t) -> p c t", c=2),
                    mb(0, 2),
                )
                pv = []
                for c in range(3):
                    pvc = pvpool.tile([128, 6 * T], bf, tag=f"pv{c}", name="pv_t")
                    ndve = 6 if c == 0 else 4
                    nc.vector.tensor_mul(
                        pvc[:, : ndve * T].rearrange("p (c t) -> p c t", c=ndve),
                        xt[:, 4 * T : (4 + ndve) * T].rearrange(
                            "p (c t) -> p c t", c=ndve
                        ),
                        mb(1 + c, ndve),
                    )
                    if ndve < 6:
                        ngp = 6 - ndve
                        nc.gpsimd.tensor_mul(
                            pvc[:, ndve * T :].rearrange("p (c t) -> p c t", c=ngp),
                            xt[:, (4 + ndve) * T : 10 * T].rearrange(
                                "p (c t) -> p c t", c=ngp
                            ),
                            mb(1 + c, ngp),
                        )
                    pv.append(pvc)

                def PS(ch):  # s-scaled chunk
                    return ps[:, ch * T : (ch + 1) * T]

                def PV(c, ch):  # v_c-scaled chunk (ch is global 4..9)
                    return pv[c][:, (ch - 4) * T : (ch - 3) * T]

                def XT(ch):  # raw chunk
                    return xt[:, ch * T : (ch + 1) * T]

                yt = ypool.tile([128, 10 * T], bf, tag="yo", name="y_t")

                def mm_accum(contribs, name):
                    p = psum.tile([128, T], f32, tag="ps", name=name, bufs=8)
                    n = len(contribs)
                    for i, (wi, rhs) in enumerate(contribs):
                        nc.tensor.matmul(
                            p[:, :], W(wi), rhs, start=(i == 0), stop=(i == n - 1)
                        )
                    return p

                # ---- 0e : dot collapsed on DVE (3 diag K-chunks -> 1) ----
                dot = cpool.tile([128, T], bf, tag="dot", name="dot_t", bufs=2)
                nc.vector.tensor_add(dot[:, :], PV(0, 7), PV(1, 8))
                nc.vector.tensor_add(dot[:, :], dot[:, :], PV(2, 9))
                for m in range(2):
                    p = mm_accum(
                        [
                            (0 * 2 + m, PS(0)),
                            (1 * 2 + m, PS(1)),
                            (2 * 2 + m, dot[:, :]),
                        ],
                        "ps0e",
                    )
                    nc.scalar.copy(out=yt[:, m * T : (m + 1) * T], in_=p[:, :])
                for m in range(2):
                    p = mm_accum(
                        [
                            (6 + 0 * 2 + m, PS(2)),
                            (6 + 1 * 2 + m, PS(3)),
                            (6 + 2 * 2 + m, PV(0, 4)),
                            (6 + 2 * 2 + m, PV(1, 5)),
                            (6 + 2 * 2 + m, PV(2, 6)),
                        ],
                        "ps0o",
                    )
                    nc.scalar.copy(out=yt[:, (2 + m) * T : (3 + m) * T], in_=p[:, :])

                # ---- 1e / 1o : g post-scaled ----
                # (wb, xg0, hb, cb, ob): weight base, g-input chunk, h-chunk
                # base (same parity as output), cross-chunk base (opposite
                # l=1 parity), output chunk base.
                for wb, xg0, hb, cb, ob in ((12, 2, 4, 7, 4), (17, 0, 7, 4, 7)):
                    # g = x0?' @ Wg  (unscaled)
                    gp = mm_accum([(wb + 0, XT(xg0)), (wb + 1, XT(xg0 + 1))], "psg")
                    sg = cpool.tile([128, T], bf, tag="sg", name="sg_t", bufs=4)
                    nc.scalar.copy(out=sg[:, :], in_=gp[:, :])
                    for c in range(3):
                        a, b = (c + 1) % 3, (c + 2) % 3
                        p = mm_accum(
                            [
                                (wb + 2, PS(hb + c)),         # h: x1par_c * s
                                (wb + 3, PV(b, cb + a)),      # k+: x1op_a * v_b
                                (wb + 4, PV(a, cb + b)),      # k-: x1op_b * v_a
                            ],
                            "ps1",
                        )
                        sc = cpool.tile([128, T], bf, tag="sc", name="sc_t", bufs=8)
                        nc.scalar.copy(out=sc[:, :], in_=p[:, :])
                        ys = yt[:, (ob + c) * T : (ob + c + 1) * T]
                        nc.vector.tensor_mul(
                            ys, mbt[:, (1 + c) * T : (2 + c) * T], sg[:, :]
                        )
                        nc.vector.tensor_add(ys, ys, sc[:, :])

                nc.sync.dma_start(out=y[t, :, :], in_=yt[:, :])
    nc.finalize()
    return nc


_PROG_CACHE = {}


def _get_program(Bs):
    if Bs not in _PROG_CACHE:
        _PROG_CACHE[Bs] = _build_program(Bs)
    return _PROG_CACHE[Bs]


def run(inputs, trace=False, **kw):
    in1 = np.asarray(inputs["in1"], np.float32)
    in2 = np.asarray(inputs["in2"], np.float32)
    B = in1.shape[0]
    assert B % (N_CORES * T) == 0, B
    Bs = B // N_CORES

    wpk = _pack_weights(
        np.asarray(inputs["W0e"], np.float32),
        np.asarray(inputs["W0o"], np.float32),
        np.asarray(inputs["W1e"], np.float32),
        np.asarray(inputs["W1o"], np.float32),
    )

    in_maps = []
    for i in range(N_CORES):
        ssl = slice(i * Bs, (i + 1) * Bs)
        xs, s4s = _prep_shard(in1[ssl], in2[ssl])
        in_maps.append({"x": xs, "s4": s4s, "w": wpk})

    nc = _get_program(Bs)
    res = run_bass_kernel_spmd(nc, in_maps, list(range(N_CORES)), trace=trace, **kw)

    out = np.empty((B, 1280), np.float32)
    for i in range(N_CORES):
        out[i * Bs : (i + 1) * Bs] = _post_shard(res.results[i]["y"])
    return out, res


def kernel(**inputs):
    out, _ = run(inputs, trace=False)
    return out


# revision 21
# speedup vs baseline: 1.4841x; 1.4841x over previous
"""Trainium2 Bass kernel for the L1 tensor-product problem.

Math (per batch row b):
  out0e = [x0e*s, CG*(x1o.v)] @ W0e * NORM0E
  out0o = [x0o*s, CG*(x1e.v)] @ W0o * NORM0O
  out1e_c = [CG*x0o*v_c, CG*x1e_c*s, CGC*cross(x1o,v)_c] @ W1e * NORM1E
  out1o_c = [CG*x0e*v_c, CG*x1o_c*s, CGC*cross(x1e,v)_c] @ W1o * NORM1O

Kernel strategy (pure data parallel over batch, 8 cores), v2:
  * Everything bf16 on the wire and on the matmul path; PSUM accumulates
    fp32; output is written bf16 and upcast on the host (rel-err budget
    2e-2, bf16 gives ~3e-3).
  * Host packs x per core as [ntiles, 128, 10*T] so each partition's
    tile-load is one contiguous 10KB DMA descriptor (the fp32 baseline
    was descriptor-bound at ~2KB/descriptor, DMA 95% busy).
  * The per-row scalars (s, v_c) commute with the feature contraction:
    the x0o@W1e / x0e@W1o blocks are computed UNSCALED (2 matmuls each
    instead of 6 pre-scaled ones) and scaled on DVE afterwards.  This
    cuts matmuls per tile from 54 to 42.
  * The 40 pre-scaled products are emitted as 4 wide DVE instructions
    (x_all * s broadcast; x1e/x1o * v_c broadcast) using stride-0
    broadcast APs; multiplier rows are partition-broadcast by GpSimd,
    keeping the PE free of the old ones-matmul broadcasts.
"""

import sys

sys.path.insert(0, "/opt/trn_rl_repo")

import numpy as np

import concourse.bass as bass
import concourse.bacc as bacc
import concourse.mybir as mybir
from concourse.bass_utils import run_bass_kernel_spmd
from concourse.tile import TileContext

N_CORES = 8
T = 512  # batch columns per tile

# irreps: 256x0e + 256x0o + 128x1e + 128x1o
CG = 1.0 / 3.0**0.5
CGC = 1.0 / 6.0**0.5
NORM0E = (1.0 / 384.0) ** 0.5
NORM0O = (1.0 / 384.0) ** 0.5
NORM1E = (3.0 / 512.0) ** 0.5
NORM1O = (3.0 / 512.0) ** 0.5

_BF16 = None


def _bf16():
    global _BF16
    if _BF16 is None:
        import ml_dtypes

        _BF16 = np.dtype(ml_dtypes.bfloat16)
    return _BF16


def _pack_weights(W0e, W0o, W1e, W1o):
    """Fold constants/signs; 22 lhsT chunks [128,128] side by side.

    Order: 0e (kc0m0,kc0m1,kc1m0,kc1m1,kc2m0,kc2m1), 0o (same 6),
    1e (g0,g1,h,k+,k-), 1o (g0,g1,h,k+,k-).
    """
    W0e = W0e.astype(np.float64) * NORM0E
    W0e[256:] *= CG
    W0o = W0o.astype(np.float64) * NORM0O
    W0o[256:] *= CG
    W1e = W1e.astype(np.float64) * NORM1E
    W1e[:384] *= CG
    W1e[384:] *= CGC
    W1o = W1o.astype(np.float64) * NORM1O
    W1o[:384] *= CG
    W1o[384:] *= CGC
    chunks = []
    for W in (W0e, W0o):  # [384, 256]
        for kc in range(3):
            for mc in range(2):
                chunks.append(W[kc * 128 : (kc + 1) * 128, mc * 128 : (mc + 1) * 128])
    for W in (W1e, W1o):  # [512, 128]
        chunks.append(W[0:128, :])      # g0
        chunks.append(W[128:256, :])    # g1
        chunks.append(W[256:384, :])    # h
        chunks.append(W[384:512, :])    # k+
        chunks.append(-W[384:512, :])   # k-
    chunks.append(np.eye(128, dtype=np.float64))  # 22: identity (combine accum)
    packed = np.concatenate(chunks, axis=1)
    return np.ascontiguousarray(packed.astype(_bf16()))


def _prep_shard(in1_s, in2_s):
    """in1 [Bs,1280] -> x [nt, 128, 10*T] bf16; in2 [Bs,4] -> s4 [nt,4,T].

    Chunk order: 0,1=x0e  2,3=x0o  4+c=x1e_c  7+c=x1o_c.
    """
    Bs = in1_s.shape[0]
    nt = Bs // T
    dt = _bf16()
    x = np.empty((nt, 128, 10, T), dt)
    x[:, :, 0:4] = in1_s[:, 0:512].reshape(nt, T, 4, 128).transpose(0, 3, 2, 1)
    x[:, :, 4:7] = in1_s[:, 512:896].reshape(nt, T, 128, 3).transpose(0, 2, 3, 1)
    x[:, :, 7:10] = in1_s[:, 896:1280].reshape(nt, T, 128, 3).transpose(0, 2, 3, 1)
    s4 = np.ascontiguousarray(in2_s.reshape(nt, T, 4).transpose(0, 2, 1).astype(dt))
    return np.ascontiguousarray(x.reshape(nt, 128, 10 * T)), s4


def _post_shard(y):
    """Device y [nt, 128, 10*T] bf16 -> [Bs, 1280] fp32 original layout."""
    nt = y.shape[0]
    y = np.asarray(y).reshape(nt, 128, 10, T).astype(np.float32)
    out = np.empty((nt, T, 1280), np.float32)
    out[:, :, 0:512] = y[:, :, 0:4].transpose(0, 3, 2, 1).reshape(nt, T, 512)
    out[:, :, 512:896] = y[:, :, 4:7].transpose(0, 3, 1, 2).reshape(nt, T, 384)
    out[:, :, 896:1280] = y[:, :, 7:10].transpose(0, 3, 1, 2).reshape(nt, T, 384)
    return out.reshape(nt * T, 1280)


def _build_program(Bs):
    assert Bs % T == 0, (Bs, T)
    nt = Bs // T
    bf = mybir.dt.bfloat16
    f32 = mybir.dt.float32

    nc = bacc.Bacc()
    x = nc.declare_dram_parameter("x", [nt, 128, 10 * T], bf, isOutput=False)
    s4 = nc.declare_dram_parameter("s4", [nt, 4, T], bf, isOutput=False)
    w = nc.declare_dram_parameter("w", [128, 23 * 128], bf, isOutput=False)
    y = nc.declare_dram_parameter("y", [nt, 128, 10 * T], bf, isOutput=True)

    with TileContext(nc) as tc:
        with (
            tc.tile_pool(name="wpool", bufs=1) as wpool,
            tc.tile_pool(name="xpool", bufs=2) as xpool,
            tc.tile_pool(name="mbpool", bufs=2) as mbpool,
            tc.tile_pool(name="pspool", bufs=2) as pspool,
            tc.tile_pool(name="pvpool", bufs=6) as pvpool,
            tc.tile_pool(name="cpool", bufs=2) as cpool,
            tc.tile_pool(name="ypool", bufs=2) as ypool,
            tc.tile_pool(name="psum", bufs=8, space="PSUM") as psum,
        ):
            wt = wpool.tile([128, 23 * 128], bf)
            nc.sync.dma_start(out=wt[:, :], in_=w[:, :])

            def W(i):
                return wt[:, i * 128 : (i + 1) * 128]

            for t in range(nt):
                # ---- loads ----
                xt = xpool.tile([128, 10 * T], bf, tag="xt", name="x_t")
                nc.sync.dma_start(out=xt[:, :], in_=x[t, :, :])
                # ---- multiplier broadcast via stride-0 DMA ----
                mbt = mbpool.tile([128, 4 * T], bf, tag="mb", name="mb_t")
                nc.sync.dma_start(
                    out=mbt[:, :].rearrange("p (c t) -> p c t", c=4),
                    in_=s4[t].unsqueeze(0).broadcast_to([128, 4, T]),
                )

                def mb(j, nch):
                    # [128, nch, T] stride-0 broadcast of multiplier row j
                    return (
                        mbt[:, j * T : (j + 1) * T]
                        .unsqueeze(1)
                        .broadcast_to([128, nch, T])
                    )

                # ---- products: 4 wide DVE ops (GpSimd triggers the power
                # throttle and runs ~1.6x over its cost model -- keep off) ----
                ps = pspool.tile([128, 10 * T], bf, tag="ps", name="ps_t")
                nc.vector.tensor_mul(
                    ps[:, :].rearrange("p (c t) -> p c t", c=10),
                    xt[:, :].rearrange("p (c t) -> p c t", c=10),
                    mb(0, 10),
                )
                pv = []
                for c in range(3):
                    pvc = pvpool.tile([128, 6 * T], bf, tag=f"pv{c}", name="pv_t")
                    nc.vector.tensor_mul(
                        pvc[:, :].rearrange("p (c t) -> p c t", c=6),
                        xt[:, 4 * T :].rearrange("p (c t) -> p c t", c=6),
                        mb(1 + c, 6),
                    )
                    pv.append(pvc)

                def PS(ch):  # s-scaled chunk
                    return ps[:, ch * T : (ch + 1) * T]

                def PV(c, ch):  # v_c-scaled chunk (ch is global 4..9)
                    return pv[c][:, (ch - 4) * T : (ch - 3) * T]

                def XT(ch):  # raw chunk
                    return xt[:, ch * T : (ch + 1) * T]

                yt = ypool.tile([128, 10 * T], bf, tag="yo", name="y_t")

                def mm_accum(contribs, name):
                    p = psum.tile([128, T], f32, tag="ps", name=name, bufs=8)
                    n = len(contribs)
                    for i, (wi, rhs) in enumerate(contribs):
                        nc.tensor.matmul(
                            p[:, :], W(wi), rhs, start=(i == 0), stop=(i == n - 1)
                        )
                    return p

                # ---- 0e / 0o : diag K-chunks collapsed to a DVE dot ----
                for base, wb, psa, dch in ((0, 0, 0, 7), (2, 6, 2, 4)):
                    dot = cpool.tile([128, T], bf, tag=f"dot{base}", name="dot_t", bufs=2)
                    nc.vector.tensor_add(dot[:, :], PV(0, dch), PV(1, dch + 1))
                    nc.vector.tensor_add(dot[:, :], dot[:, :], PV(2, dch + 2))
                    for m in range(2):
                        p = mm_accum(
                            [
                                (wb + 0 * 2 + m, PS(psa)),
                                (wb + 1 * 2 + m, PS(psa + 1)),
                                (wb + 2 * 2 + m, dot[:, :]),
                            ],
                            "ps0",
                        )
                        nc.scalar.copy(
                            out=yt[:, (base + m) * T : (base + m + 1) * T], in_=p[:, :]
                        )

                # ---- 1e / 1o : g post-scaled, combine via identity matmul ----
                # (wb, xg0, hb, cb, ob): weight base, g-input chunk, h-chunk
                # base (same parity as output), cross-chunk base (opposite
                # l=1 parity), output chunk base.
                for wb, xg0, hb, cb, ob in ((12, 2, 4, 7, 4), (17, 0, 7, 4, 7)):
                    # g = x0?' @ Wg  (unscaled)
                    gp = mm_accum([(wb + 0, XT(xg0)), (wb + 1, XT(xg0 + 1))], "psg")
                    sg = cpool.tile([128, T], bf, tag="sg", name="sg_t", bufs=4)
                    nc.scalar.copy(out=sg[:, :], in_=gp[:, :])
                    # t3[c] = v_c * g for all 3 components in one DVE op
                    t3 = cpool.tile([128, 3 * T], bf, tag="t3", name="t3_t", bufs=4)
                    nc.vector.tensor_mul(
                        t3[:, :].rearrange("p (c t) -> p c t", c=3),
                        mbt[:, T:].rearrange("p (c t) -> p c t", c=3),
                        sg[:, :].unsqueeze(1).broadcast_to([128, 3, T]),
                    )
                    for c in range(3):
                        a, b = (c + 1) % 3, (c + 2) % 3
                        p = mm_accum(
                            [
                                (wb + 2, PS(hb + c)),         # h: x1par_c * s
                                (wb + 3, PV(b, cb + a)),      # k+: x1op_a * v_b
                                (wb + 4, PV(a, cb + b)),      # k-: x1op_b * v_a
                                (22, t3[:, c * T : (c + 1) * T]),  # += v_c * g
                            ],
                            "ps1",
                        )
                        nc.scalar.copy(
                            out=yt[:, (ob + c) * T : (ob + c + 1) * T], in_=p[:, :]
                        )

                nc.sync.dma_start(out=y[t, :, :], in_=yt[:, :])
    nc.finalize()
    return nc


_PROG_CACHE = {}


def _get_program(Bs):
    if Bs not in _PROG_CACHE:
        _PROG_CACHE[Bs] = _build_program(Bs)
    return _PROG_CACHE[Bs]


def run(inputs, trace=False, **kw):
    in1 = np.asarray(inputs["in1"], np.float32)
    in2 = np.asarray(inputs["in2"], np.float32)
    B = in1.shape[0]
    assert B % (N_CORES * T) == 0, B
    Bs = B // N_CORES

    wpk = _pack_weights(
        np.asarray(inputs["W0e"], np.float32),
        np.asarray(inputs["W0o"], np.float32),
        np.asarray(inputs["W1e"], np.float32),
        np.asarray(inputs["W1o"], np.float32),
    )

    in_maps = []
    for i in range(N_CORES):
        ssl = slice(i * Bs, (i + 1) * Bs)
        xs, s4s = _prep_shard(in1[ssl], in2[ssl])
        in_maps.append({"x": xs, "s4": s4s, "w": wpk})

    nc = _get_program(Bs)
    res = run_bass_kernel_spmd(nc, in_maps, list(range(N_CORES)), trace=trace, **kw)

    out = np.empty((B, 1280), np.float32)
    for i in range(N_CORES):
        out[i * Bs : (i + 1) * Bs] = _post_shard(res.results[i]["y"])
    return out, res


def kernel(**inputs):
    out, _ = run(inputs, trace=False)
    return out
